# revision 1
# baseline (speedup 1.0000x reference)
"""GATv2 2-layer classifier on 8 Trainium2 NeuronCores (Bass/Tile).

Sharding (per spec hint): nodes sharded contiguously across 8 cores; edges
partitioned by destination core and sorted/grouped by 128-dst blocks so
segment-softmax and scatter-add stay local; source-side features all-gathered
(halo exchange) before each conv layer; weights replicated.

Per-layer device algorithm (per core):
  xlal = x_loc @ [Wl | wal],  wal[:,h] = 0.2*sum_c Wl[:,h*C+c]*att[h,c]
  AllGather xlal -> xlal_full          (only the src side needs the halo)
  xral = x_loc @ [Wr | war]            (dst side is local)
  per 128-edge tile:
    gather g_l = xlal_full[src], g_r = xral[dst_local]     (indirect DMA)
    z = g_l + g_r                      (cols HC..HC+H hold 0.2*linear part)
    absred_h = sum_c 0.8*att[h,c]*relu(z[h*C+c])  (fused scalar_tensor_tensor)
    score = absred + z[:, HC:HC+H]     (lrelu(t) = 0.2 t + 0.8 relu(t))
    e = exp(score)                     (scores bounded ~+-7; no segment max)
    v = [e*xl | e]
    psum[d,:] += sum_e onehot(dst)[e,d] * v[e,:]   (TensorE scatter-add)
  per 128-dst block: out = psum[:, :HC]/psum[:, HC:] + b, relu
"""

import os

import ml_dtypes
import numpy as np

import concourse.bacc as bacc
import concourse.bass as bass
import concourse.mybir as mybir
import concourse.tile as tile
from concourse.bass import IndirectOffsetOnAxis
from concourse.bass_utils import run_bass_kernel_spmd
from concourse.masks import make_identity

P = 128
M_CORES = 8
F32 = mybir.dt.float32
BF16 = mybir.dt.bfloat16
I32 = mybir.dt.int32
AL = mybir.AluOpType
AF = mybir.ActivationFunctionType

LAST_RESULTS = None  # BassKernelResults from the most recent kernel() call


# ----------------------------------------------------------------------------
# host-side edge sharding
# ----------------------------------------------------------------------------

def _shard_edges(src, dst, n, nloc):
    """Partition edges by dst core, sort by dst, group into 128-dst blocks.
    Pad each block's run to a multiple of 128 using the per-block max tile
    count across cores, so all 8 cores share one program shape."""
    nblk = (nloc + P - 1) // P
    per_core = []
    for k in range(M_CORES):
        m = (dst >= k * nloc) & (dst < (k + 1) * nloc)
        s, d = src[m], dst[m] - k * nloc
        order = np.argsort(d, kind="stable")
        per_core.append((s[order], d[order]))

    tiles_per_blk = np.zeros(nblk, dtype=np.int64)
    for k in range(M_CORES):
        _, d = per_core[k]
        cnt = np.bincount(d // P, minlength=nblk)
        tiles_per_blk = np.maximum(tiles_per_blk, (cnt + P - 1) // P)
    ntiles = int(tiles_per_blk.sum())
    tile_start = np.concatenate([[0], np.cumsum(tiles_per_blk)])[:-1]

    srcs = np.zeros((M_CORES, ntiles * P), dtype=np.int32)
    dstl = np.zeros((M_CORES, ntiles * P), dtype=np.int32)
    dstb = np.full((M_CORES, ntiles * P), -1.0, dtype=np.float32)
    for k in range(M_CORES):
        s, d = per_core[k]
        cnt = np.bincount(d // P, minlength=nblk)
        ofs = np.concatenate([[0], np.cumsum(cnt)])[:-1]
        for b in range(nblk):
            nb = cnt[b]
            p0 = tile_start[b] * P
            srcs[k, p0 : p0 + nb] = s[ofs[b] : ofs[b] + nb]
            dstl[k, p0 : p0 + nb] = d[ofs[b] : ofs[b] + nb]
            dstb[k, p0 : p0 + nb] = (d[ofs[b] : ofs[b] + nb] - b * P).astype(np.float32)

    # pack [ntiles*P] -> [P, ntiles] (tile t = column t)
    def pack(a):
        return np.ascontiguousarray(a.reshape(ntiles, P).T)

    return ([pack(srcs[k]) for k in range(M_CORES)],
            [pack(dstl[k]) for k in range(M_CORES)],
            [pack(dstb[k]) for k in range(M_CORES)],
            tiles_per_blk.astype(int).tolist(), nblk)


def _aug(W, att, heads, ch):
    """[W | 0.2 * per-head (W . att)] -> [in, heads*ch + heads]"""
    wa = 0.2 * (W.reshape(W.shape[0], heads, ch) * att[None]).sum(-1)
    return np.concatenate([W, wa], axis=1).astype(np.float32)


# ----------------------------------------------------------------------------
# device program
# ----------------------------------------------------------------------------

def _edge_phase(nc, tc, pools, tabs, cfg, layer):
    """Edge loop + per-block finalize for one conv layer."""
    (const, idx, work, psum_big, psum_tr, psum_lg) = pools
    heads, ch = cfg[f"H{layer}"], cfg["C"]
    HC = heads * ch
    W = HC + heads
    nblk, tiles_per_blk, nloc = cfg["nblk"], cfg["tiles_per_blk"], cfg["nloc"]
    full_tab = tabs[f"xlal{layer}_full"]
    loc_tab = tabs[f"xral{layer}_loc"]
    src_sb, dstl_sb, dstb_sb = tabs["src_sb"], tabs["dstl_sb"], tabs["dstb_sb"]
    att_sb = tabs[f"att04_{layer}"]
    bias_sb = tabs[f"b{layer}"]
    iota_bf, ident_bf = tabs["iota_bf"], tabs["ident_bf"]

    t_idx = 0
    for b in range(nblk):
        rows = min(P, nloc - b * P)
        psum_acc = psum_big.tile([P, W], F32, tag="psum_acc", name=f"acc{layer}_{b}")
        for t in range(tiles_per_blk[b]):
            col = t_idx
            t_idx += 1
            g_l = work.tile([P, W], BF16, tag="g_l", name=f"gl{layer}_{col}")
            g_r = work.tile([P, W], BF16, tag="g_r", name=f"gr{layer}_{col}")
            nc.gpsimd.indirect_dma_start(
                out=g_l[:], out_offset=None, in_=full_tab[:],
                in_offset=IndirectOffsetOnAxis(ap=src_sb[:, col : col + 1], axis=0))
            nc.gpsimd.indirect_dma_start(
                out=g_r[:], out_offset=None, in_=loc_tab[:],
                in_offset=IndirectOffsetOnAxis(ap=dstl_sb[:, col : col + 1], axis=0))
            z = work.tile([P, W], BF16, tag="z", name=f"z{layer}_{col}")
            nc.vector.tensor_tensor(out=z[:], in0=g_l[:], in1=g_r[:], op=AL.add)
            q = work.tile([P, HC], BF16, tag="q", name=f"q{layer}_{col}")
            absred = work.tile([P, heads], F32, tag="absred", name=f"ar{layer}_{col}")
            for h in range(heads):
                sl = slice(h * ch, (h + 1) * ch)
                nc.vector.scalar_tensor_tensor(
                    out=q[:, sl], in0=z[:, sl], scalar=0.0, in1=att_sb[:, sl],
                    op0=AL.max, op1=AL.mult,
                    accum_out=absred[:, h : h + 1])
            score = work.tile([P, heads], F32, tag="score", name=f"sc{layer}_{col}")
            nc.vector.tensor_tensor(
                out=score[:], in0=absred[:], in1=z[:, HC : HC + heads], op=AL.add)
            v = work.tile([P, W], BF16, tag="v", name=f"v{layer}_{col}")
            exp_s = work.tile([P, heads], F32, tag="exp_s", name=f"e{layer}_{col}")
            nc.scalar.activation(out=exp_s[:], in_=score[:], func=AF.Exp)
            nc.vector.tensor_copy(out=v[:, HC : HC + heads], in_=exp_s[:])
            for h in range(heads):
                sl = slice(h * ch, (h + 1) * ch)
                nc.vector.tensor_scalar(
                    out=v[:, sl], in0=g_l[:, sl],
                    scalar1=exp_s[:, h : h + 1], scalar2=None, op0=AL.mult)
            m_bf = work.tile([P, P], BF16, tag="m_bf", name=f"m{layer}_{col}")
            nc.vector.tensor_scalar(
                out=m_bf[:], in0=iota_bf[:],
                scalar1=dstb_sb[:, col : col + 1], scalar2=None, op0=AL.is_equal)
            nc.tensor.matmul(
                psum_acc[:], lhsT=m_bf[:], rhs=v[:],
                start=(t == 0), stop=(t == tiles_per_blk[b] - 1))

        # ---- block finalize ----
        rec = work.tile([P, heads], F32, tag="rec", name=f"rec{layer}_{b}")
        nc.vector.reciprocal(out=rec[:], in_=psum_acc[:, HC : HC + heads])
        h_sb = work.tile([P, HC], BF16, tag="h_sb", name=f"h{layer}_{b}")
        for h in range(heads):
            sl = slice(h * ch, (h + 1) * ch)
            nc.vector.scalar_tensor_tensor(
                out=h_sb[:, sl], in0=psum_acc[:, sl], scalar=rec[:, h : h + 1],
                in1=bias_sb[:, sl], op0=AL.mult, op1=AL.add)
        nc.vector.tensor_scalar(
            out=h_sb[:], in0=h_sb[:], scalar1=0.0, scalar2=None, op0=AL.max)

        if layer == 1:
            # transpose h block -> hT_dram[256, nloc] for layer-2 matmuls
            for half in range(HC // P):
                pt = psum_tr.tile([P, P], BF16, tag="pt", name=f"pt{b}_{half}")
                nc.tensor.transpose(
                    out=pt[:, :rows], in_=h_sb[:rows, half * P : (half + 1) * P],
                    identity=ident_bf[:rows, :rows])
                ht_t = work.tile([P, P], BF16, tag="ht_t", name=f"ht{b}_{half}")
                nc.vector.tensor_copy(out=ht_t[:, :rows], in_=pt[:, :rows])
                nc.sync.dma_start(
                    out=tabs["hT_dram"][half * P : (half + 1) * P,
                                        b * P : b * P + rows],
                    in_=ht_t[:, :rows])
        else:
            # final linear fused into block finalize
            pt2 = psum_tr.tile([P, P], BF16, tag="pt", name=f"pt2_{b}")
            nc.tensor.transpose(out=pt2[:ch, :rows], in_=h_sb[:rows, :ch],
                                identity=ident_bf[:rows, :rows])
            zt = work.tile([ch, P], BF16, tag="zt", name=f"zt{b}")
            nc.vector.tensor_copy(out=zt[:, :rows], in_=pt2[:ch, :rows])
            pl = psum_lg.tile([P, cfg["NC"]], F32, tag="pl", name=f"pl{b}")
            nc.tensor.matmul(pl[:rows, :], lhsT=zt[:, :rows], rhs=tabs["wlin_sb"][:],
                             start=True, stop=True)
            o_sb = work.tile([P, cfg["NC"]], F32, tag="o_sb", name=f"o{b}")
            nc.vector.scalar_tensor_tensor(
                out=o_sb[:rows, :], in0=pl[:rows, :], scalar=1.0,
                in1=tabs["blin_sb"][:rows, :], op0=AL.mult, op1=AL.add)
            nc.sync.dma_start(out=tabs["out_dram"][b * P : b * P + rows, :],
                              in_=o_sb[:rows, :])


def _dense_phase(nc, tc, pools, tabs, cfg, layer):
    """x_loc @ [Wl|wal] and x_loc @ [Wr|war]; rows stored to DRAM tables."""
    (const, idx, work, psum_big, psum_tr, psum_lg) = pools
    nloc, nblk = cfg["nloc"], cfg["nblk"]
    kin = cfg["IN"] if layer == 1 else cfg["H1"] * cfg["C"]
    heads = cfg[f"H{layer}"]
    W = heads * cfg["C"] + heads
    nk = kin // P
    lhsT = tabs["xT_sb"] if layer == 1 else tabs["hT_sb"]  # list of [P, nloc]
    for m in range(nblk):
        rows = min(P, nloc - m * P)
        for side in ("l", "r"):
            wt = tabs[f"W{side}{layer}a_sb"]  # list of [P, W]
            ps = psum_big.tile([P, W], F32, tag="psum_acc", name=f"d{layer}{side}{m}")
            for k in range(nk):
                nc.tensor.matmul(
                    ps[:rows, :], lhsT=lhsT[k][:, m * P : m * P + rows],
                    rhs=wt[k][:], start=(k == 0), stop=(k == nk - 1))
            ob = work.tile([P, W], BF16, tag="dense_o", name=f"do{layer}{side}{m}")
            nc.vector.tensor_copy(out=ob[:rows, :], in_=ps[:rows, :])
            dst_tab = tabs[f"x{side}al{layer}_loc" if side == "r" else f"xlal{layer}_loc"]
            nc.sync.dma_start(out=dst_tab[m * P : m * P + rows, :], in_=ob[:rows, :])


def _build(cfg):
    """Build the full 8-core SPMD program. Returns (nc, names of inputs)."""
    nc = bacc.Bacc("TRN2", target_bir_lowering=False, debug=False,
                   enable_asserts=False, num_devices=M_CORES)
    nloc, ntiles = cfg["nloc"], cfg["ntiles"]
    IN, C, H1, H2, NC = cfg["IN"], cfg["C"], cfg["H1"], cfg["H2"], cfg["NC"]
    W1, W2 = H1 * C + H1, H2 * C + H2
    n_full = cfg["N"]

    ins = {}
    def ext_in(name, shape, dt):
        ins[name] = nc.dram_tensor(name, list(shape), dt, kind="ExternalInput").ap()
        return ins[name]

    xT_in = ext_in("xT", (IN, nloc), F32)
    wl1_in = ext_in("Wl1a", (IN, W1), F32)
    wr1_in = ext_in("Wr1a", (IN, W1), F32)
    att1_in = ext_in("att04_1", (P, H1 * C), F32)
    b1_in = ext_in("b1bc", (P, H1 * C), F32)
    wl2_in = ext_in("Wl2a", (H1 * C, W2), BF16)
    wr2_in = ext_in("Wr2a", (H1 * C, W2), BF16)
    att2_in = ext_in("att04_2", (P, H2 * C), F32)
    b2_in = ext_in("b2bc", (P, H2 * C), F32)
    wlin_in = ext_in("Wlin", (C, NC), BF16)
    blin_in = ext_in("blinbc", (P, NC), F32)
    src_in = ext_in("src_idx", (P, ntiles), I32)
    dstl_in = ext_in("dstl_idx", (P, ntiles), I32)
    dstb_in = ext_in("dstblk", (P, ntiles), F32)
    out_dram = nc.dram_tensor("out", [nloc, NC], F32, kind="ExternalOutput").ap()

    with tile.TileContext(nc) as tc:
        const = tc.alloc_tile_pool(name="const", bufs=1)
        idx = tc.alloc_tile_pool(name="idx", bufs=1)
        work = tc.alloc_tile_pool(name="work", bufs=3)
        psum_big = tc.alloc_tile_pool(name="psum_big", bufs=2, space="PSUM")
        psum_tr = tc.alloc_tile_pool(name="psum_tr", bufs=2, space="PSUM")
        psum_lg = tc.alloc_tile_pool(name="psum_lg", bufs=2, space="PSUM")
        dram = tc.alloc_tile_pool(name="dram", bufs=1, space="DRAM")
        pools = (const, idx, work, psum_big, psum_tr, psum_lg)

        tabs = {"out_dram": out_dram}

        # --- constants / index tables into SBUF ---
        def load_const(name, ap_in, shape, dt, pool=None):
            t = (pool or const).tile(list(shape), dt, name=name)
            nc.sync.dma_start(out=t[:], in_=ap_in[:])
            return t

        tabs["src_sb"] = load_const("src_sb", src_in, (P, ntiles), I32, idx)
        tabs["dstl_sb"] = load_const("dstl_sb", dstl_in, (P, ntiles), I32, idx)
        tabs["dstb_sb"] = load_const("dstb_sb", dstb_in, (P, ntiles), F32, idx)
        tabs["att04_1"] = load_const("att04_1sb", att1_in, (P, H1 * C), F32)
        tabs["b1"] = load_const("b1sb", b1_in, (P, H1 * C), F32)
        tabs["att04_2"] = load_const("att04_2sb", att2_in, (P, H2 * C), F32)
        tabs["b2"] = load_const("b2sb", b2_in, (P, H2 * C), F32)
        tabs["wlin_sb"] = load_const("wlin_sb", wlin_in, (C, NC), BF16)
        tabs["blin_sb"] = load_const("blin_sb", blin_in, (P, NC), F32)
        tabs["Wl1a_sb"] = [load_const(f"wl1a{k}", wl1_in[k * P : (k + 1) * P, :],
                                      (P, W1), F32) for k in range(IN // P)]
        tabs["Wr1a_sb"] = [load_const(f"wr1a{k}", wr1_in[k * P : (k + 1) * P, :],
                                      (P, W1), F32) for k in range(IN // P)]
        tabs["Wl2a_sb"] = [load_const(f"wl2a{k}", wl2_in[k * P : (k + 1) * P, :],
                                      (P, W2), BF16) for k in range(H1 * C // P)]
        tabs["Wr2a_sb"] = [load_const(f"wr2a{k}", wr2_in[k * P : (k + 1) * P, :],
                                      (P, W2), BF16) for k in range(H1 * C // P)]
        tabs["xT_sb"] = [load_const(f"xT{k}", xT_in[k * P : (k + 1) * P, :],
                                    (P, nloc), F32) for k in range(IN // P)]

        it32 = const.tile([P, P], I32, name="iota_i32")
        nc.gpsimd.iota(it32[:], pattern=[[1, P]], base=0, channel_multiplier=0)
        iota_bf = const.tile([P, P], BF16, name="iota_bf")
        nc.vector.tensor_copy(out=iota_bf[:], in_=it32[:])
        tabs["iota_bf"] = iota_bf
        ident_bf = const.tile([P, P], BF16, name="ident_bf")
        make_identity(nc, ident_bf[:])
        tabs["ident_bf"] = ident_bf

        # --- DRAM scratch ---
        tabs["xlal1_loc"] = dram.tile([nloc, W1], BF16, name="xlal1_loc")
        tabs["xral1_loc"] = dram.tile([nloc, W1], BF16, name="xral1_loc")
        tabs["xlal1_full"] = dram.tile([n_full, W1], BF16, name="xlal1_full",
                                       addr_space="Shared")
        tabs["hT_dram"] = dram.tile([H1 * C, nloc], BF16, name="hT_dram")
        tabs["xlal2_loc"] = dram.tile([nloc, W2], BF16, name="xlal2_loc")
        tabs["xral2_loc"] = dram.tile([nloc, W2], BF16, name="xral2_loc")
        tabs["xlal2_full"] = dram.tile([n_full, W2], BF16, name="xlal2_full",
                                       addr_space="Shared")

        rg = [list(range(M_CORES))]

        # ---- layer 1 ----
        _dense_phase(nc, tc, pools, tabs, cfg, 1)
        nc.gpsimd.collective_compute(
            "AllGather", AL.bypass, replica_groups=rg,
            ins=[tabs["xlal1_loc"][:].opt()], outs=[tabs["xlal1_full"][:].opt()])
        _edge_phase(nc, tc, pools, tabs, cfg, 1)

        # ---- layer 2 ----
        tabs["hT_sb"] = [load_const(f"hT{k}", tabs["hT_dram"][k * P : (k + 1) * P, :],
                                    (P, nloc), BF16) for k in range(H1 * C // P)]
        _dense_phase(nc, tc, pools, tabs, cfg, 2)
        nc.gpsimd.collective_compute(
            "AllGather", AL.bypass, replica_groups=rg,
            ins=[tabs["xlal2_loc"][:].opt()], outs=[tabs["xlal2_full"][:].opt()])
        _edge_phase(nc, tc, pools, tabs, cfg, 2)

        for p in (dram, psum_lg, psum_tr, psum_big, work, idx, const):
            p.release()

    nc.compile()
    return nc


# ----------------------------------------------------------------------------
# entry point
# ----------------------------------------------------------------------------

def _prep(inputs):
    x = np.asarray(inputs["x"], dtype=np.float32)
    ei = np.asarray(inputs["edge_index"])
    N, IN = x.shape
    E = ei.shape[1]
    Wl1 = np.asarray(inputs["Wl1"], np.float32)
    Wr1 = np.asarray(inputs["Wr1"], np.float32)
    att1 = np.asarray(inputs["att1"], np.float32)
    b1 = np.asarray(inputs["b1"], np.float32)
    Wl2 = np.asarray(inputs["Wl2"], np.float32)
    Wr2 = np.asarray(inputs["Wr2"], np.float32)
    att2 = np.asarray(inputs["att2"], np.float32)
    b2 = np.asarray(inputs["b2"], np.float32)
    Wlin = np.asarray(inputs["Wlin"], np.float32)
    blin = np.asarray(inputs["blin"], np.float32)
    H1, C = att1.shape
    H2 = att2.shape[0]
    NC = Wlin.shape[1]
    nloc = N // M_CORES

    loop = np.arange(N, dtype=np.int64)
    src = np.concatenate([ei[0].astype(np.int64), loop])
    dst = np.concatenate([ei[1].astype(np.int64), loop])
    srcs, dstls, dstbs, tiles_per_blk, nblk = _shard_edges(src, dst, N, nloc)

    cfg = dict(N=N, IN=IN, C=C, H1=H1, H2=H2, NC=NC, nloc=nloc, nblk=nblk,
               tiles_per_blk=tiles_per_blk, ntiles=sum(tiles_per_blk))

    Wl1a, Wr1a = _aug(Wl1, att1, H1, C), _aug(Wr1, att1, H1, C)
    Wl2a, Wr2a = _aug(Wl2, att2, H2, C), _aug(Wr2, att2, H2, C)
    bf = ml_dtypes.bfloat16
    shared = dict(
        Wl1a=Wl1a, Wr1a=Wr1a,
        att04_1=np.broadcast_to(0.8 * att1.reshape(1, -1), (P, H1 * C)).copy(),
        b1bc=np.broadcast_to(b1.reshape(1, -1), (P, H1 * C)).copy(),
        Wl2a=Wl2a.astype(bf), Wr2a=Wr2a.astype(bf),
        att04_2=np.broadcast_to(0.8 * att2.reshape(1, -1), (P, H2 * C)).copy(),
        b2bc=np.broadcast_to(b2.reshape(1, -1), (P, H2 * C)).copy(),
        Wlin=Wlin.astype(bf),
        blinbc=np.broadcast_to(blin.reshape(1, -1), (P, NC)).copy(),
    )
    in_maps = []
    for k in range(M_CORES):
        m = dict(shared)
        m["xT"] = np.ascontiguousarray(x[k * nloc : (k + 1) * nloc].T)
        m["src_idx"] = srcs[k]
        m["dstl_idx"] = dstls[k]
        m["dstblk"] = dstbs[k]
        in_maps.append(m)
    return cfg, in_maps


def kernel(**inputs):
    global LAST_RESULTS
    cfg, in_maps = _prep(inputs)
    nc = _build(cfg)
    res = run_bass_kernel_spmd(
        nc, in_maps, core_ids=list(range(M_CORES)),
        trace=bool(os.environ.get("BASS_TRACE")))
    LAST_RESULTS = res
    out = np.concatenate([res.results[k]["out"] for k in range(M_CORES)], axis=0)
    return out.astype(np.float32)



# revision 5
# speedup vs baseline: 1.0014x; 1.0014x over previous
"""GATv2 2-layer classifier on 8 Trainium2 NeuronCores (Bass/Tile).

Sharding (per spec hint): nodes sharded contiguously across 8 cores; edges
partitioned by destination core and sorted/grouped by 128-dst blocks so
segment-softmax and scatter-add stay local; source-side features all-gathered
(halo exchange) before each conv layer; weights replicated.

Per-layer device algorithm (per core):
  xlal = x_loc @ [Wl | wal],  wal[:,h] = 0.2*sum_c Wl[:,h*C+c]*att[h,c]
  AllGather xlal -> xlal_full          (only the src side needs the halo)
  xral = x_loc @ [Wr | war]            (dst side is local)
  per 128-edge tile:
    gather g_l = xlal_full[src], g_r = xral[dst_local]     (indirect DMA)
    z = g_l + g_r                      (cols HC..HC+H hold 0.2*linear part)
    absred_h = sum_c 0.8*att[h,c]*relu(z[h*C+c])  (fused scalar_tensor_tensor)
    score = absred + z[:, HC:HC+H]     (lrelu(t) = 0.2 t + 0.8 relu(t))
    e = exp(score)                     (scores bounded ~+-7; no segment max)
    v = [e*xl | e]
    psum[d,:] += sum_e onehot(dst)[e,d] * v[e,:]   (TensorE scatter-add)
  per 128-dst block: out = psum[:, :HC]/psum[:, HC:] + b, relu
"""

import os

import ml_dtypes
import numpy as np

import concourse.bacc as bacc
import concourse.bass as bass
import concourse.mybir as mybir
import concourse.tile as tile
from concourse.bass import IndirectOffsetOnAxis
from concourse.bass_utils import run_bass_kernel_spmd
from concourse.masks import make_identity

P = 128
M_CORES = 8
F32 = mybir.dt.float32
BF16 = mybir.dt.bfloat16
I32 = mybir.dt.int32
AL = mybir.AluOpType
AF = mybir.ActivationFunctionType

LAST_RESULTS = None  # BassKernelResults from the most recent kernel() call


# ----------------------------------------------------------------------------
# host-side edge sharding
# ----------------------------------------------------------------------------

def _shard_edges(src, dst, n, nloc):
    """Partition edges by dst core, sort by dst, group into 128-dst blocks.
    Pad each block's run to a multiple of 128 using the per-block max tile
    count across cores, so all 8 cores share one program shape."""
    nblk = (nloc + P - 1) // P
    per_core = []
    for k in range(M_CORES):
        m = (dst >= k * nloc) & (dst < (k + 1) * nloc)
        s, d = src[m], dst[m] - k * nloc
        order = np.argsort(d, kind="stable")
        per_core.append((s[order], d[order]))

    tiles_per_blk = np.zeros(nblk, dtype=np.int64)
    for k in range(M_CORES):
        _, d = per_core[k]
        cnt = np.bincount(d // P, minlength=nblk)
        tiles_per_blk = np.maximum(tiles_per_blk, (cnt + P - 1) // P)
    ntiles = int(tiles_per_blk.sum())
    tile_start = np.concatenate([[0], np.cumsum(tiles_per_blk)])[:-1]

    srcs = np.zeros((M_CORES, ntiles * P), dtype=np.int32)
    dstl = np.zeros((M_CORES, ntiles * P), dtype=np.int32)
    dstb = np.full((M_CORES, ntiles * P), -1.0, dtype=np.float32)
    for k in range(M_CORES):
        s, d = per_core[k]
        cnt = np.bincount(d // P, minlength=nblk)
        ofs = np.concatenate([[0], np.cumsum(cnt)])[:-1]
        for b in range(nblk):
            nb = cnt[b]
            p0 = tile_start[b] * P
            srcs[k, p0 : p0 + nb] = s[ofs[b] : ofs[b] + nb]
            dstl[k, p0 : p0 + nb] = d[ofs[b] : ofs[b] + nb]
            dstb[k, p0 : p0 + nb] = (d[ofs[b] : ofs[b] + nb] - b * P).astype(np.float32)

    # pack [ntiles*P] -> [P, ntiles] (tile t = column t)
    def pack(a):
        return np.ascontiguousarray(a.reshape(ntiles, P).T)

    return ([pack(srcs[k]) for k in range(M_CORES)],
            [pack(dstl[k]) for k in range(M_CORES)],
            [pack(dstb[k]) for k in range(M_CORES)],
            tiles_per_blk.astype(int).tolist(), nblk)


def _aug(W, att, heads, ch):
    """[W | 0.2 * per-head (W . att)] -> [in, heads*ch + heads]"""
    wa = 0.2 * (W.reshape(W.shape[0], heads, ch) * att[None]).sum(-1)
    return np.concatenate([W, wa], axis=1).astype(np.float32)


# ----------------------------------------------------------------------------
# device program
# ----------------------------------------------------------------------------

def _edge_phase(nc, tc, pools, tabs, cfg, layer):
    """Edge loop + per-block finalize for one conv layer."""
    (const, idx, work, psum_big, psum_tr, psum_lg) = pools
    heads, ch = cfg[f"H{layer}"], cfg["C"]
    HC = heads * ch
    W = HC + heads
    nblk, tiles_per_blk, nloc = cfg["nblk"], cfg["tiles_per_blk"], cfg["nloc"]
    full_tab = tabs[f"xlal{layer}_full"]
    loc_tab = tabs[f"xral{layer}_loc"]
    src_sb, dstl_sb, dstb_sb = tabs["src_sb"], tabs["dstl_sb"], tabs["dstb_sb"]
    att_sb = tabs[f"att04_{layer}"]
    bias_sb = tabs[f"b{layer}"]
    iota_bf, ident_bf = tabs["iota_bf"], tabs["ident_bf"]

    t_idx = 0
    for b in range(nblk):
        rows = min(P, nloc - b * P)
        psum_acc = psum_big.tile([P, W], F32, tag="psum_acc", name=f"acc{layer}_{b}")
        for t in range(tiles_per_blk[b]):
            col = t_idx
            t_idx += 1
            g_l = work.tile([P, W], BF16, tag="g_l", name=f"gl{layer}_{col}")
            g_r = work.tile([P, W], BF16, tag="g_r", name=f"gr{layer}_{col}")
            nc.gpsimd.indirect_dma_start(
                out=g_l[:], out_offset=None, in_=full_tab[:],
                in_offset=IndirectOffsetOnAxis(ap=src_sb[:, col : col + 1], axis=0))
            nc.gpsimd.indirect_dma_start(
                out=g_r[:], out_offset=None, in_=loc_tab[:],
                in_offset=IndirectOffsetOnAxis(ap=dstl_sb[:, col : col + 1], axis=0))
            z = work.tile([P, W], BF16, tag="z", name=f"z{layer}_{col}")
            nc.vector.tensor_tensor(out=z[:], in0=g_l[:], in1=g_r[:], op=AL.add)
            q = work.tile([P, HC], BF16, tag="q", name=f"q{layer}_{col}")
            absred = work.tile([P, heads], F32, tag="absred", name=f"ar{layer}_{col}")
            for h in range(heads):
                sl = slice(h * ch, (h + 1) * ch)
                nc.vector.scalar_tensor_tensor(
                    out=q[:, sl], in0=z[:, sl], scalar=0.0, in1=att_sb[:, sl],
                    op0=AL.max, op1=AL.mult,
                    accum_out=absred[:, h : h + 1])
            score = work.tile([P, heads], F32, tag="score", name=f"sc{layer}_{col}")
            nc.vector.tensor_tensor(
                out=score[:], in0=absred[:], in1=z[:, HC : HC + heads], op=AL.add)
            v = work.tile([P, W], BF16, tag="v", name=f"v{layer}_{col}")
            exp_s = work.tile([P, heads], F32, tag="exp_s", name=f"e{layer}_{col}")
            nc.scalar.activation(out=exp_s[:], in_=score[:], func=AF.Exp)
            nc.vector.tensor_copy(out=v[:, HC : HC + heads], in_=exp_s[:])
            for h in range(heads):
                sl = slice(h * ch, (h + 1) * ch)
                nc.vector.tensor_scalar(
                    out=v[:, sl], in0=g_l[:, sl],
                    scalar1=exp_s[:, h : h + 1], scalar2=None, op0=AL.mult)
            m_bf = work.tile([P, P], BF16, tag="m_bf", name=f"m{layer}_{col}")
            nc.vector.tensor_scalar(
                out=m_bf[:], in0=iota_bf[:],
                scalar1=dstb_sb[:, col : col + 1], scalar2=None, op0=AL.is_equal)
            nc.tensor.matmul(
                psum_acc[:], lhsT=m_bf[:], rhs=v[:],
                start=(t == 0), stop=(t == tiles_per_blk[b] - 1))

        # ---- block finalize ----
        rec = work.tile([P, heads], F32, tag="rec", name=f"rec{layer}_{b}")
        nc.vector.reciprocal(out=rec[:], in_=psum_acc[:, HC : HC + heads])
        h_sb = work.tile([P, HC], BF16, tag="h_sb", name=f"h{layer}_{b}")
        for h in range(heads):
            sl = slice(h * ch, (h + 1) * ch)
            nc.vector.scalar_tensor_tensor(
                out=h_sb[:, sl], in0=psum_acc[:, sl], scalar=rec[:, h : h + 1],
                in1=bias_sb[:, sl], op0=AL.mult, op1=AL.add)
        nc.vector.tensor_scalar(
            out=h_sb[:], in0=h_sb[:], scalar1=0.0, scalar2=None, op0=AL.max)

        if layer == 1:
            # transpose h block -> hT_dram[256, nloc] for layer-2 matmuls
            for half in range(HC // P):
                pt = psum_tr.tile([P, P], BF16, tag="pt", name=f"pt{b}_{half}")
                nc.tensor.transpose(
                    out=pt[:, :rows], in_=h_sb[:rows, half * P : (half + 1) * P],
                    identity=ident_bf[:rows, :rows])
                ht_t = work.tile([P, P], BF16, tag="ht_t", name=f"ht{b}_{half}")
                nc.vector.tensor_copy(out=ht_t[:, :rows], in_=pt[:, :rows])
                nc.sync.dma_start(
                    out=tabs["hT_dram"][half * P : (half + 1) * P,
                                        b * P : b * P + rows],
                    in_=ht_t[:, :rows])
        else:
            # final linear fused into block finalize
            pt2 = psum_tr.tile([P, P], BF16, tag="pt", name=f"pt2_{b}")
            nc.tensor.transpose(out=pt2[:ch, :rows], in_=h_sb[:rows, :ch],
                                identity=ident_bf[:rows, :rows])
            zt = work.tile([ch, P], BF16, tag="zt", name=f"zt{b}")
            nc.vector.tensor_copy(out=zt[:, :rows], in_=pt2[:ch, :rows])
            pl = psum_lg.tile([P, cfg["NC"]], F32, tag="pl", name=f"pl{b}")
            nc.tensor.matmul(pl[:rows, :], lhsT=zt[:, :rows], rhs=tabs["wlin_sb"][:],
                             start=True, stop=True)
            o_sb = work.tile([P, cfg["NC"]], F32, tag="o_sb", name=f"o{b}")
            nc.vector.scalar_tensor_tensor(
                out=o_sb[:rows, :], in0=pl[:rows, :], scalar=1.0,
                in1=tabs["blin_sb"][:rows, :], op0=AL.mult, op1=AL.add)
            nc.sync.dma_start(out=tabs["out_dram"][b * P : b * P + rows, :],
                              in_=o_sb[:rows, :])


def _dense_phase(nc, tc, pools, tabs, cfg, layer):
    """x_loc @ [Wl|wal] and x_loc @ [Wr|war]; rows stored to DRAM tables."""
    (const, idx, work, psum_big, psum_tr, psum_lg) = pools
    nloc, nblk = cfg["nloc"], cfg["nblk"]
    kin = cfg["IN"] if layer == 1 else cfg["H1"] * cfg["C"]
    heads = cfg[f"H{layer}"]
    W = heads * cfg["C"] + heads
    nk = kin // P
    lhsT = tabs["xT_sb"] if layer == 1 else tabs["hT_sb"]  # list of [P, nloc]
    for m in range(nblk):
        rows = min(P, nloc - m * P)
        for side in ("l", "r"):
            wt = tabs[f"W{side}{layer}a_sb"]  # list of [P, W]
            ps = psum_big.tile([P, W], F32, tag="psum_acc", name=f"d{layer}{side}{m}")
            for k in range(nk):
                nc.tensor.matmul(
                    ps[:rows, :], lhsT=lhsT[k][:, m * P : m * P + rows],
                    rhs=wt[k][:], start=(k == 0), stop=(k == nk - 1))
            ob = work.tile([P, W], BF16, tag="dense_o", name=f"do{layer}{side}{m}")
            nc.vector.tensor_copy(out=ob[:rows, :], in_=ps[:rows, :])
            dst_tab = tabs[f"x{side}al{layer}_loc" if side == "r" else f"xlal{layer}_loc"]
            nc.sync.dma_start(out=dst_tab[m * P : m * P + rows, :], in_=ob[:rows, :])


def _build(cfg):
    """Build the full 8-core SPMD program. Returns (nc, names of inputs)."""
    nc = bacc.Bacc("TRN2", target_bir_lowering=False, debug=False,
                   enable_asserts=False, num_devices=M_CORES)
    nloc, ntiles = cfg["nloc"], cfg["ntiles"]
    IN, C, H1, H2, NC = cfg["IN"], cfg["C"], cfg["H1"], cfg["H2"], cfg["NC"]
    W1, W2 = H1 * C + H1, H2 * C + H2
    n_full = cfg["N"]

    ins = {}
    def ext_in(name, shape, dt):
        ins[name] = nc.dram_tensor(name, list(shape), dt, kind="ExternalInput").ap()
        return ins[name]

    xT_in = ext_in("xT", (IN, nloc), F32)
    wl1_in = ext_in("Wl1a", (IN, W1), F32)
    wr1_in = ext_in("Wr1a", (IN, W1), F32)
    att1_in = ext_in("att04_1", (P, H1 * C), F32)
    b1_in = ext_in("b1bc", (P, H1 * C), F32)
    wl2_in = ext_in("Wl2a", (H1 * C, W2), BF16)
    wr2_in = ext_in("Wr2a", (H1 * C, W2), BF16)
    att2_in = ext_in("att04_2", (P, H2 * C), F32)
    b2_in = ext_in("b2bc", (P, H2 * C), F32)
    wlin_in = ext_in("Wlin", (C, NC), BF16)
    blin_in = ext_in("blinbc", (P, NC), F32)
    src_in = ext_in("src_idx", (P, ntiles), I32)
    dstl_in = ext_in("dstl_idx", (P, ntiles), I32)
    dstb_in = ext_in("dstblk", (P, ntiles), F32)
    out_dram = nc.dram_tensor("out", [nloc, NC], F32, kind="ExternalOutput").ap()

    with tile.TileContext(nc) as tc:
        const = tc.alloc_tile_pool(name="const", bufs=1)
        idx = tc.alloc_tile_pool(name="idx", bufs=1)
        work = tc.alloc_tile_pool(name="work", bufs=3)
        psum_big = tc.alloc_tile_pool(name="psum_big", bufs=2, space="PSUM")
        psum_tr = tc.alloc_tile_pool(name="psum_tr", bufs=2, space="PSUM")
        psum_lg = tc.alloc_tile_pool(name="psum_lg", bufs=2, space="PSUM")
        dram = tc.alloc_tile_pool(name="dram", bufs=1, space="DRAM")
        pools = (const, idx, work, psum_big, psum_tr, psum_lg)

        tabs = {"out_dram": out_dram}

        # --- constants / index tables into SBUF ---
        def load_const(name, ap_in, shape, dt, pool=None):
            t = (pool or const).tile(list(shape), dt, name=name)
            nc.sync.dma_start(out=t[:], in_=ap_in[:])
            return t

        tabs["src_sb"] = load_const("src_sb", src_in, (P, ntiles), I32, idx)
        tabs["dstl_sb"] = load_const("dstl_sb", dstl_in, (P, ntiles), I32, idx)
        tabs["dstb_sb"] = load_const("dstb_sb", dstb_in, (P, ntiles), F32, idx)
        tabs["att04_1"] = load_const("att04_1sb", att1_in, (P, H1 * C), F32)
        tabs["b1"] = load_const("b1sb", b1_in, (P, H1 * C), F32)
        tabs["att04_2"] = load_const("att04_2sb", att2_in, (P, H2 * C), F32)
        tabs["b2"] = load_const("b2sb", b2_in, (P, H2 * C), F32)
        tabs["wlin_sb"] = load_const("wlin_sb", wlin_in, (C, NC), BF16)
        tabs["blin_sb"] = load_const("blin_sb", blin_in, (P, NC), F32)
        tabs["Wl1a_sb"] = [load_const(f"wl1a{k}", wl1_in[k * P : (k + 1) * P, :],
                                      (P, W1), F32) for k in range(IN // P)]
        tabs["Wr1a_sb"] = [load_const(f"wr1a{k}", wr1_in[k * P : (k + 1) * P, :],
                                      (P, W1), F32) for k in range(IN // P)]
        tabs["Wl2a_sb"] = [load_const(f"wl2a{k}", wl2_in[k * P : (k + 1) * P, :],
                                      (P, W2), BF16) for k in range(H1 * C // P)]
        tabs["Wr2a_sb"] = [load_const(f"wr2a{k}", wr2_in[k * P : (k + 1) * P, :],
                                      (P, W2), BF16) for k in range(H1 * C // P)]
        tabs["xT_sb"] = [load_const(f"xT{k}", xT_in[k * P : (k + 1) * P, :],
                                    (P, nloc), F32) for k in range(IN // P)]

        it32 = const.tile([P, P], I32, name="iota_i32")
        nc.gpsimd.iota(it32[:], pattern=[[1, P]], base=0, channel_multiplier=0)
        iota_bf = const.tile([P, P], BF16, name="iota_bf")
        nc.vector.tensor_copy(out=iota_bf[:], in_=it32[:])
        tabs["iota_bf"] = iota_bf
        ident_bf = const.tile([P, P], BF16, name="ident_bf")
        make_identity(nc, ident_bf[:])
        tabs["ident_bf"] = ident_bf

        # --- DRAM scratch ---
        tabs["xlal1_loc"] = dram.tile([nloc, W1], BF16, name="xlal1_loc")
        tabs["xral1_loc"] = dram.tile([nloc, W1], BF16, name="xral1_loc")
        tabs["xlal1_full"] = dram.tile([n_full, W1], BF16, name="xlal1_full",
                                       addr_space="Shared")
        tabs["hT_dram"] = dram.tile([H1 * C, nloc], BF16, name="hT_dram")
        tabs["xlal2_loc"] = dram.tile([nloc, W2], BF16, name="xlal2_loc")
        tabs["xral2_loc"] = dram.tile([nloc, W2], BF16, name="xral2_loc")
        tabs["xlal2_full"] = dram.tile([n_full, W2], BF16, name="xlal2_full",
                                       addr_space="Shared")

        rg = [list(range(M_CORES))]

        # ---- layer 1 ----
        _dense_phase(nc, tc, pools, tabs, cfg, 1)
        nc.gpsimd.collective_compute(
            "AllGather", AL.bypass, replica_groups=rg,
            ins=[tabs["xlal1_loc"][:].opt()], outs=[tabs["xlal1_full"][:].opt()])
        _edge_phase(nc, tc, pools, tabs, cfg, 1)

        # ---- layer 2 ----
        tabs["hT_sb"] = [load_const(f"hT{k}", tabs["hT_dram"][k * P : (k + 1) * P, :],
                                    (P, nloc), BF16) for k in range(H1 * C // P)]
        _dense_phase(nc, tc, pools, tabs, cfg, 2)
        nc.gpsimd.collective_compute(
            "AllGather", AL.bypass, replica_groups=rg,
            ins=[tabs["xlal2_loc"][:].opt()], outs=[tabs["xlal2_full"][:].opt()])
        _edge_phase(nc, tc, pools, tabs, cfg, 2)

        for p in (dram, psum_lg, psum_tr, psum_big, work, idx, const):
            p.release()

    nc.compile()
    return nc


# ----------------------------------------------------------------------------
# entry point
# ----------------------------------------------------------------------------

def _prep(inputs):
    x = np.asarray(inputs["x"], dtype=np.float32)
    ei = np.asarray(inputs["edge_index"])
    N, IN = x.shape
    E = ei.shape[1]
    Wl1 = np.asarray(inputs["Wl1"], np.float32)
    Wr1 = np.asarray(inputs["Wr1"], np.float32)
    att1 = np.asarray(inputs["att1"], np.float32)
    b1 = np.asarray(inputs["b1"], np.float32)
    Wl2 = np.asarray(inputs["Wl2"], np.float32)
    Wr2 = np.asarray(inputs["Wr2"], np.float32)
    att2 = np.asarray(inputs["att2"], np.float32)
    b2 = np.asarray(inputs["b2"], np.float32)
    Wlin = np.asarray(inputs["Wlin"], np.float32)
    blin = np.asarray(inputs["blin"], np.float32)
    H1, C = att1.shape
    H2 = att2.shape[0]
    NC = Wlin.shape[1]
    nloc = N // M_CORES

    loop = np.arange(N, dtype=np.int64)
    src = np.concatenate([ei[0].astype(np.int64), loop])
    dst = np.concatenate([ei[1].astype(np.int64), loop])
    srcs, dstls, dstbs, tiles_per_blk, nblk = _shard_edges(src, dst, N, nloc)

    cfg = dict(N=N, IN=IN, C=C, H1=H1, H2=H2, NC=NC, nloc=nloc, nblk=nblk,
               tiles_per_blk=tiles_per_blk, ntiles=sum(tiles_per_blk))

    Wl1a, Wr1a = _aug(Wl1, att1, H1, C), _aug(Wr1, att1, H1, C)
    Wl2a, Wr2a = _aug(Wl2, att2, H2, C), _aug(Wr2, att2, H2, C)
    bf = ml_dtypes.bfloat16
    shared = dict(
        Wl1a=Wl1a, Wr1a=Wr1a,
        att04_1=np.broadcast_to(0.8 * att1.reshape(1, -1), (P, H1 * C)).copy(),
        b1bc=np.broadcast_to(b1.reshape(1, -1), (P, H1 * C)).copy(),
        Wl2a=Wl2a.astype(bf), Wr2a=Wr2a.astype(bf),
        att04_2=np.broadcast_to(0.8 * att2.reshape(1, -1), (P, H2 * C)).copy(),
        b2bc=np.broadcast_to(b2.reshape(1, -1), (P, H2 * C)).copy(),
        Wlin=Wlin.astype(bf),
        blinbc=np.broadcast_to(blin.reshape(1, -1), (P, NC)).copy(),
    )
    in_maps = []
    for k in range(M_CORES):
        m = dict(shared)
        m["xT"] = np.ascontiguousarray(x[k * nloc : (k + 1) * nloc].T)
        m["src_idx"] = srcs[k]
        m["dstl_idx"] = dstls[k]
        m["dstblk"] = dstbs[k]
        in_maps.append(m)
    return cfg, in_maps


def kernel(**inputs):
    global LAST_RESULTS
    cfg, in_maps = _prep(inputs)
    nc = _build(cfg)
    res = run_bass_kernel_spmd(
        nc, in_maps, core_ids=list(range(M_CORES)),
        trace=bool(os.environ.get("BASS_TRACE")))
    LAST_RESULTS = res
    out = np.concatenate([res.results[k]["out"] for k in range(M_CORES)], axis=0)
    return out.astype(np.float32)
